# revision 61
# baseline (speedup 1.0000x reference)
"""Distributed Trainium2 Bass kernel for nn_Attention_15479062135535.

Reference computation (B=4, N=2048, DIM=512, H=8, D=64):
    qkv = x @ w_qkv; q,k,v = split(qkv)
    sim = (q * D**-.5) @ k^T + pos_bias
    masked batches (focus_present_mask): sim = -inf off-diagonal -> attn == eye
    out = softmax(sim) @ v @ w_out

Sharding (8 cores, no collectives):
  - Batches NOT fully masked ("attention batches", list U): each core owns one
    batch's contiguous block of NQ_U query rows x all 8 heads. K/V are
    recomputed per core from the full x[b] (cheap vs attention).
  - Fully masked batches (list S): attn == identity exactly, so
    out = (x @ w_v) @ w_out row-wise. Rows of all S batches are concatenated
    and split evenly across cores as a cheap passthrough stage.
  - |U| must divide 8; odd mask counts are rounded down (the leftover masked
    batch goes through the attention path with an eye-encoded bias, which is
    exact as well).

Numerics:
  - All matmuls bf16 with f32 PSUM accumulation.
  - softmax without max subtraction (|sim| <~ 8, exp is safe in f32).
  - exp(sim + bias) = exp(sim) * exp(bias): exp(pos_bias) is precomputed on
    host in f32 and passed as bf16; for masked rows it is the exact 0/1
    one-hot, so identity attention is exact.
  - softmax denominator: a ones-column appended to V makes the AV matmul
    accumulate sum(exp) in PSUM partition 64 for free.

The single SPMD program is specialized at build time on (NQ_U, NQ_M) only;
all per-core differences are in the input data.
"""

import sys

sys.path.insert(0, "/opt/trn_rl_repo")

import numpy as np
import ml_dtypes

B, N, DIM = 4, 2048, 512
H, D = 8, 64
HID = H * D
P = 128
NCORES = 8

BF16 = ml_dtypes.bfloat16

LEGALIZE = True  # sim_check sets False (CoreSim rejects the synthetic NoOps)
USE_AG = False  # AllGather K/V sharing: correctness+perf negative on this runtime

_NC_CACHE = {}


def _legalize_waits(nc, max_waits=1):
    """Split multi-wait sync_info into standalone NoOp waits.

    The walrus build in this container supports only one sync-wait command
    per instruction ("Too many sync wait commands" in setupSyncWait), while
    Tile embeds the full wait list in each instruction. Hoisting the extra
    waits onto engine-tagged NoOps immediately before the instruction is
    semantically identical (the engine stalls on each in turn).
    """
    import concourse.mybir as mybir
    import bass_rust

    ctr = 0
    for fn in nc.m.functions:
        for blk in fn.blocks:
            out = []
            changed = False
            for inst in blk.instructions:
                si = inst.sync_info
                if si is not None and si.on_wait and len(si.on_wait) > max_waits:
                    waits = list(si.on_wait)
                    for w in waits[:-max_waits]:
                        ctr += 1
                        nop = mybir.InstNoOp(name=f"waitnop-{ctr}")
                        nop.engine = inst.engine
                        nop.sync_info = bass_rust.SyncInfo(on_wait=[w], on_update=[])
                        out.append(nop)
                    inst.sync_info = bass_rust.SyncInfo(
                        on_wait=waits[-max_waits:], on_update=list(si.on_update)
                    )
                    changed = True
                out.append(inst)
            if changed:
                blk.instructions = out
    return ctr


def _build(NQ_U: int, NQ_M: int, cpb: int = 0, use_ag: bool = False):
    """Build the uniform SPMD Bass program for the given per-core row counts."""
    import concourse.bass as bass
    import concourse.mybir as mybir
    import concourse.tile as tile

    f32 = mybir.dt.float32
    bf16 = mybir.dt.bfloat16
    Exp = mybir.ActivationFunctionType.Exp

    nc = bass.Bass()

    w_qkv = nc.declare_dram_parameter("w_qkv", (P, 4, 3 * HID), bf16, isOutput=False)
    w_out8 = nc.declare_dram_parameter("w_out8", (H, D, DIM), bf16, isOutput=False)

    if NQ_U:
        QBS = min(512, NQ_U)  # q-block size (free dim of sim matmuls)
        NQB = NQ_U // QBS
        KCP = 4  # key-chunks per psum tile / exp batch
        use_ag = use_ag and cpb >= 2
        if not use_ag:
            xT = nc.declare_dram_parameter("xT", (P, 4, N), bf16, isOutput=False)
        xqT = nc.declare_dram_parameter("xqT", (P, 4, NQ_U), bf16, isOutput=False)
        # ebT = exp(pos_bias)^T slabs: exp(sim+bias) = exp(sim)*exp(bias)
        ebT = nc.declare_dram_parameter(
            "ebT", (H, NQB, N, QBS), bf16, isOutput=False
        )
        # eye8[r, r*64:(r+1)*64] == 1: lhsT selector that broadcasts row r of
        # an (R, n) rhs onto 64 output partitions
        R8 = H * NQB
        eye8 = nc.declare_dram_parameter("eye8", (R8, R8 * D), bf16, isOutput=False)
        out_u = nc.declare_dram_parameter("out_u", (NQ_U, DIM), f32, isOutput=True)
    if NQ_M:
        MBS = 256  # masked-stage column chunk (NQ_M is a multiple of 256)
        xmT = nc.declare_dram_parameter("xmT", (P, 4, NQ_M), bf16, isOutput=False)
        w_out4 = nc.declare_dram_parameter("w_out4", (P, 4, DIM), bf16, isOutput=False)
        wv4T = nc.declare_dram_parameter("wv4T", (P, 4, DIM), bf16, isOutput=False)
        out_mT = nc.declare_dram_parameter("out_mT", (DIM, NQ_M), f32, isOutput=True)

    with tile.TileContext(nc) as tc:
        with (
            tc.tile_pool(name="const", bufs=1) as const,
            tc.tile_pool(name="expbp", bufs=2) as expbp,
            tc.tile_pool(name="attnp", bufs=4) as attnp,
            tc.tile_pool(name="outp", bufs=3) as outp,
            tc.tile_pool(name="small", bufs=3) as small,
            tc.tile_pool(name="ps_p", bufs=3, space="PSUM") as ps_big,
            tc.tile_pool(name="ps_o", bufs=2, space="PSUM") as ps_o,
        ):
            ps_sim = ps_big  # shared pool: all tiles are one PSUM slot size
            # ---- weights: column-chunked DMAs so consumers start early ----
            wq_sb = const.tile([P, 4, 3 * HID], bf16, name="wq_sb")

            def load_wq(c0, c1):
                nc.sync.dma_start(
                    wq_sb[:, :, c0 * 128 : c1 * 128], w_qkv[:, :, c0 * 128 : c1 * 128]
                )

            if NQ_U:
                # ---- stage A: activations ----
                xqT_sb = const.tile([P, 4, NQ_U], bf16, name="xqT_sb")
                nc.sync.dma_start(xqT_sb[:], xqT[:])
                if use_ag:
                    load_wq(4, 12)  # k+v columns (KV shards need them first)
                    load_wq(0, 4)  # q columns
                else:
                    load_wq(0, 4)  # q columns
                    load_wq(4, 8)  # k columns
                    xT_sb = const.tile([P, 4, N], bf16, name="xT_sb")
                    for n in range(4):
                        nc.gpsimd.dma_start(
                            xT_sb[:, :, n * 512 : (n + 1) * 512],
                            xT[:, :, n * 512 : (n + 1) * 512],
                        )
                    load_wq(8, 12)  # v columns
                eye8_sb = const.tile([R8, R8 * D], bf16, name="eye8_sb")
                nc.sync.dma_start(eye8_sb[:], eye8[:])
            else:
                load_wq(0, 12)

            wo8_sb = []
            for h in range(H):
                t = const.tile([D, DIM], bf16, name=f"wo8_{h}")
                nc.sync.dma_start(t[:], w_out8[h])
                wo8_sb.append(t)

            if NQ_M:
                # ---- passthrough: out_m = x_m @ (w_v @ w_out). W2 = w_v@w_out
                # is computed once on device; out_m groups are emitted as
                # fillers inside stage C so their matmuls use PE idle time ----
                xmT_sb = const.tile([P, 4, NQ_M], bf16, name="xmT_sb")
                nc.sync.dma_start(xmT_sb[:], xmT[:])
                wo4_sb = const.tile([P, 4, DIM], bf16, name="wo4_sb")
                nc.sync.dma_start(wo4_sb[:], w_out4[:])
                wv4T_sb = const.tile([P, 4, DIM], bf16, name="wv4T_sb")
                nc.sync.dma_start(wv4T_sb[:], wv4T[:])
                w2_sb = const.tile([P, 4, DIM], bf16, name="w2_sb")
                MCHUNKS = []
                o = 0
                while o < NQ_M:
                    w = min(512, NQ_M - o)
                    MCHUNKS.append((o, w))
                    o += w
                NMC = len(MCHUNKS)

                def emit_w2(dc):
                    # W2[dc-chunk] = w_v @ w_out (contraction over hid):
                    # lhsT = w_v^T chunk (hid, d), rhs = w_out (hid, out)
                    ps = ps_big.tile([P, 512], f32, tag="ps")
                    for kc in range(4):
                        nc.tensor.matmul(
                            ps[:],
                            wv4T_sb[:, kc, dc * 128 : (dc + 1) * 128],
                            wo4_sb[:, kc, :],
                            start=(kc == 0),
                            stop=(kc == 3),
                        )
                    nc.vector.tensor_copy(w2_sb[:, dc, :], ps[:])

                def emit_masked_out(mo, n):
                    o, w = MCHUNKS[n]
                    nsl = slice(o, o + w)
                    ps = ps_big.tile([P, 512], f32, tag="ps")
                    for kc in range(4):
                        nc.tensor.matmul(
                            ps[:, :w],
                            w2_sb[:, kc, mo * 128 : (mo + 1) * 128],
                            xmT_sb[:, kc, nsl],
                            start=(kc == 0),
                            stop=(kc == 3),
                        )
                    ot = outp.tile([P, 512], f32, tag="otm")
                    nc.vector.tensor_copy(ot[:, :w], ps[:, :w])
                    nc.sync.dma_start(out_mT[mo * 128 : (mo + 1) * 128, nsl], ot[:, :w])

            if NQ_U:
                # ---- stage B: q^T, k^T (d on partitions, head pairs stacked),
                # v (+ones col); emitted partly as fillers inside stage C ----
                qT2 = [const.tile([P, NQ_U], bf16, name=f"qT2_{hp}") for hp in range(4)]
                kT2 = [const.tile([P, N], bf16, name=f"kT2_{hp}") for hp in range(4)]
                v_sb = const.tile([P, 16, H, D + 1], bf16, name="v_sb")
                if not use_ag:
                    nc.vector.memset(v_sb[:, :, :, D], 1.0)

                def emit_Q(mq):
                    for n in range(NQ_U // QBS):
                        ps = ps_big.tile([P, 512], f32, tag="ps")
                        for kc in range(4):
                            nc.tensor.matmul(
                                ps[:, :QBS],
                                wq_sb[:, kc, mq * 128 : (mq + 1) * 128],
                                xqT_sb[:, kc, n * QBS : (n + 1) * QBS],
                                start=(kc == 0),
                                stop=(kc == 3),
                            )
                        nc.scalar.copy(qT2[mq][:, n * QBS : (n + 1) * QBS], ps[:, :QBS])

                def emit_K(mk, n):
                    ps = ps_big.tile([P, 512], f32, tag="ps")
                    for kc in range(4):
                        nc.tensor.matmul(
                            ps[:],
                            wq_sb[:, kc, HID + mk * 128 : HID + (mk + 1) * 128],
                            xT_sb[:, kc, n * 512 : (n + 1) * 512],
                            start=(kc == 0),
                            stop=(kc == 3),
                        )
                    if n % 2 == 0:
                        nc.scalar.copy(kT2[mk][:, n * 512 : (n + 1) * 512], ps[:])
                    else:
                        nc.vector.tensor_copy(kT2[mk][:, n * 512 : (n + 1) * 512], ps[:])

                def emit_V(t):
                    ps = ps_big.tile([P, 512], f32, tag="ps")
                    for kc in range(4):
                        nc.tensor.matmul(
                            ps[:],
                            xT_sb[:, kc, t * 128 : (t + 1) * 128],
                            wq_sb[:, kc, 2 * HID : 3 * HID],
                            start=(kc == 0),
                            stop=(kc == 3),
                        )
                    nc.vector.tensor_copy(
                        v_sb[:, t, :, 0:D],
                        ps[:].rearrange("p (h d) -> p h d", h=H),
                    )

                if use_ag:
                    # ---- KV shards + AllGather: each core computes K/V for
                    # its own x slice (== its q rows) and the cpb cores of a
                    # batch group exchange shards via AllGather ----
                    NKV = N // cpb  # kv columns produced per core
                    TKV = NKV // 128
                    VW = H * (D + 1)
                    KSZ = 4 * P * NKV
                    VSZ = TKV * P * VW
                    with tc.tile_pool(name="dramp", bufs=1, space="DRAM") as dramp:
                        ag_in = dramp.tile([KSZ + VSZ], bf16, name="ag_in")
                        ag_out = dramp.tile([cpb * (KSZ + VSZ)], bf16, name="ag_out")
                        kstage = const.tile([P, 4, NKV], bf16, name="kstage")
                        vstage = const.tile([P, TKV, VW], bf16, name="vstage")
                        nc.vector.memset(
                            vstage[:].rearrange("p t (h e) -> p t h e", h=H)[
                                :, :, :, D
                            ],
                            1.0,
                        )
                        for hp in range(4):
                            for n0 in range(0, NKV, 512):
                                nw = min(512, NKV - n0)
                                ps = ps_big.tile([P, 512], f32, tag="ps")
                                for kc in range(4):
                                    nc.tensor.matmul(
                                        ps[:, :nw],
                                        wq_sb[:, kc, HID + hp * 128 : HID + (hp + 1) * 128],
                                        xqT_sb[:, kc, n0 : n0 + nw],
                                        start=(kc == 0),
                                        stop=(kc == 3),
                                    )
                                nc.scalar.copy(
                                    kstage[:, hp, n0 : n0 + nw], ps[:, :nw]
                                )
                        for t in range(TKV):
                            ps = ps_big.tile([P, 512], f32, tag="ps")
                            for kc in range(4):
                                nc.tensor.matmul(
                                    ps[:],
                                    xqT_sb[:, kc, t * 128 : (t + 1) * 128],
                                    wq_sb[:, kc, 2 * HID : 3 * HID],
                                    start=(kc == 0),
                                    stop=(kc == 3),
                                )
                            nc.vector.tensor_copy(
                                vstage[:, t].rearrange("p (h e) -> p h e", h=H)[
                                    :, :, 0:D
                                ],
                                ps[:].rearrange("p (h d) -> p h d", h=H),
                            )
                        k_dst = ag_in[0:KSZ].rearrange(
                            "(hp r) -> hp r", hp=4
                        ).rearrange("hp (p q) -> p hp q", p=P)
                        assert tuple(k_dst.shape) == (P, 4, NKV), k_dst.shape
                        nc.sync.dma_start(k_dst, kstage[:])
                        v_dst = ag_in[KSZ:].rearrange(
                            "(t r) -> t r", t=TKV
                        ).rearrange("t (p f) -> p t f", p=P)
                        assert tuple(v_dst.shape) == (P, TKV, VW), v_dst.shape
                        nc.sync.dma_start(v_dst, vstage[:])
                        groups = [
                            [g * cpb + i for i in range(cpb)]
                            for g in range(NCORES // cpb)
                        ]
                        nc.gpsimd.collective_compute(
                            "AllGather",
                            mybir.AluOpType.bypass,
                            replica_groups=groups,
                            ins=[ag_in.opt()],
                            outs=[ag_out.opt()],
                        )
                        agv = ag_out[:].rearrange("(c x) -> c x", c=cpb)
                        for hp in range(4):
                            src_k = agv[
                                :, hp * P * NKV : (hp + 1) * P * NKV
                            ].rearrange("c (p q) -> c p q", p=P)
                            assert tuple(src_k.shape) == (cpb, P, NKV), src_k.shape
                            k_gd = kT2[hp][:].rearrange("p (c q) -> c p q", q=NKV)
                            assert tuple(k_gd.shape) == (cpb, P, NKV), k_gd.shape
                            nc.sync.dma_start(k_gd, src_k)
                        v_flat = v_sb[:].rearrange("p t h e -> p t (h e)")
                        for c in range(cpb):
                            off = c * (KSZ + VSZ) + KSZ
                            v_src = ag_out[off : off + VSZ].rearrange(
                                "(t r) -> t r", t=TKV
                            ).rearrange("t (p f) -> p t f", p=P)
                            assert tuple(v_src.shape) == (P, TKV, VW), v_src.shape
                            nc.sync.dma_start(
                                v_flat[:, c * TKV : (c + 1) * TKV, :], v_src
                            )
                    emit_Q(0)
                else:
                    emit_Q(0)
                    for n in range(4):
                        emit_K(0, n)
                    for t in range(16):
                        emit_V(t)

                # fillers: (deadline_head, fn, args) — kq fillers must be
                # emitted before the head that consumes them; masked fillers
                # have no deadline
                fillers = []
                for hp in range(1, 4):
                    fillers.append((2 * hp, emit_Q, (hp,)))
                    if not use_ag:
                        for n in range(4):
                            fillers.append((2 * hp, emit_K, (hp, n)))
                if NQ_M:
                    for mo in range(4):
                        fillers.append((99, emit_w2, (mo,)))
                    for n in range(NMC):
                        for mo in range(4):
                            fillers.append((99, emit_masked_out, (mo, n)))
                fillers.reverse()  # pop from the end

                # ---- stage C: attention per (head, q-block) ----
                # simT psum = q@k^T; exp on the scalar engine straight out of
                # PSUM; attn = exp(sim)*exp(bias) on the vector engine; AV
                # matmul with a ones-column in V accumulates the softmax
                # denominator for free.
                aoU = [const.tile([D, NQ_U], bf16, name=f"aoU{h}") for h in range(H)]
                aoT = [const.tile([D, NQ_U], bf16, name=f"aoT{h}") for h in range(H)]
                den8 = const.tile([H * NQB, QBS], f32, name="den8")
                NKP = 16 // KCP  # psum tiles per (h, qb)
                for h in range(H):
                    hp, hl = h // 2, (h % 2) * D
                    while fillers and fillers[-1][0] <= h + 2:
                        _, f, a = fillers.pop()
                        f(*a)
                    for qb in range(NQB):
                        qsl = slice(qb * QBS, (qb + 1) * QBS)
                        slab = expbp.tile([P, 16, QBS], bf16, tag="slab")
                        nc.sync.dma_start(
                            slab[:],
                            ebT[h, qb].rearrange("(kc p) q -> p kc q", p=P),
                        )
                        ps_o_t = ps_o.tile([D + 1, QBS], f32, tag="ps_o")
                        for kp in range(NKP):
                            ps_t = ps_sim.tile([P, KCP, QBS], f32, tag="ps")
                            for i in range(KCP):
                                kc = kp * KCP + i
                                nc.tensor.matmul(
                                    ps_t[:, i, :],
                                    kT2[hp][hl : hl + D, kc * 128 : (kc + 1) * 128],
                                    qT2[hp][hl : hl + D, qsl],
                                    start=True,
                                    stop=True,
                                )
                            at = attnp.tile([P, KCP, QBS], bf16, tag="at")
                            nc.scalar.activation(at[:], ps_t[:], Exp)
                            atm = attnp.tile([P, KCP, QBS], bf16, tag="atm")
                            nc.vector.tensor_mul(
                                atm[:], at[:], slab[:, kp * KCP : (kp + 1) * KCP, :]
                            )
                            for i in range(KCP):
                                kc = kp * KCP + i
                                nc.tensor.matmul(
                                    ps_o_t[:],
                                    v_sb[:, kc, h, :],
                                    atm[:, i, :],
                                    start=(kc == 0),
                                    stop=(kc == 15),
                                )
                            if kp % 2 == 0 and len(fillers) > 8:
                                _, f, a = fillers.pop()
                                f(*a)
                        nc.scalar.copy(aoU[h][:, qsl], ps_o_t[0:D, :])
                        r = h * NQB + qb
                        d1 = small.tile([1, QBS], f32, tag="d1")
                        nc.vector.tensor_copy(d1[:], ps_o_t[D : D + 1, :])
                        nc.gpsimd.dma_start(den8[r : r + 1, :], d1[:])

                for _ in range(min(5, len(fillers))):
                    _, f, a = fillers.pop()
                    f(*a)

                # ---- batched softmax normalization ----
                recip8 = small.tile([H * NQB, QBS], f32, tag="recip8")
                nc.vector.reciprocal(recip8[:], den8[:])
                recip8_bf = small.tile([H * NQB, QBS], bf16, tag="recip8b")
                nc.vector.tensor_copy(recip8_bf[:], recip8[:])
                for h in range(H):
                    for qb in range(NQB):
                        qsl = slice(qb * QBS, (qb + 1) * QBS)
                        r = h * NQB + qb
                        ps_bt = ps_o.tile([D + 1, QBS], f32, tag="ps_o")
                        nc.tensor.matmul(
                            ps_bt[0:D, :],
                            eye8_sb[:, r * D : (r + 1) * D],
                            recip8_bf[:],
                            start=True,
                            stop=True,
                        )
                        rb = small.tile([D, QBS], f32, tag="rb")
                        nc.scalar.copy(rb[:], ps_bt[0:D, :])
                        nc.vector.tensor_mul(aoT[h][:, qsl], aoU[h][:, qsl], rb[:])
                        if fillers:
                            _, f, a = fillers.pop()
                            f(*a)

                while fillers:
                    _, f, a = fillers.pop()
                    f(*a)

                # ---- stage D: out projection ----
                for t in range(NQ_U // 128):
                    ps = ps_big.tile([P, 512], f32, tag="ps")
                    for h in range(H):
                        nc.tensor.matmul(
                            ps[:],
                            aoT[h][:, t * 128 : (t + 1) * 128],
                            wo8_sb[h][:],
                            start=(h == 0),
                            stop=(h == 7),
                        )
                    ot = outp.tile([P, 512], f32)
                    nc.vector.tensor_copy(ot[:], ps[:])
                    nc.sync.dma_start(out_u[t * 128 : (t + 1) * 128, :], ot[:])

            if NQ_M and not NQ_U:
                for mo in range(4):
                    emit_w2(mo)
                for n in range(NMC):
                    for mo in range(4):
                        emit_masked_out(mo, n)

    return nc


def _get_nc(NQ_U: int, NQ_M: int, cpb: int = 0, use_ag: bool = False,
            legalize: bool = True):
    key = (NQ_U, NQ_M, cpb, use_ag, legalize)
    if key not in _NC_CACHE:
        nc = _build(NQ_U, NQ_M, cpb=cpb, use_ag=use_ag)
        if legalize:
            _legalize_waits(nc)
        _NC_CACHE[key] = nc
    return _NC_CACHE[key]


def _to_bf16(a):
    return np.ascontiguousarray(np.asarray(a, dtype=np.float32)).astype(BF16)


def _kchunked(a2d):
    """(512, n) -> (128, 4, n): contraction dim split into 4 partition chunks."""
    k, n = a2d.shape
    assert k == 4 * P
    return np.ascontiguousarray(a2d.reshape(4, P, n).transpose(1, 0, 2))


def plan(mask):
    """Return (U, S, NQ_U, NQ_M, assignments) for the given bool mask."""
    mask = np.asarray(mask).astype(bool)
    masked = [b for b in range(B) if mask[b]]
    unmasked = [b for b in range(B) if not mask[b]]
    # |U| must divide 8 and be in {1,2,4}: move masked batches back into the
    # attention path (exact via eye-encoded bias) until it does.
    S = list(masked)
    U = list(unmasked)
    while len(U) not in (0, 1, 2, 4):
        U.append(S.pop())
    if len(U) == 0 and len(S) == 0:  # unreachable, B=4
        raise AssertionError
    NQ_U = (N * len(U)) // NCORES if U else 0
    NQ_M = (N * len(S)) // NCORES
    return U, S, NQ_U, NQ_M


def prepare(x, pos_bias, focus_present_mask, w_qkv, w_out):
    """Build (nc, in_maps, core_meta, plan_info) for the given inputs."""
    x = np.asarray(x, dtype=np.float32)
    pos_bias = np.asarray(pos_bias, dtype=np.float32)
    mask = np.asarray(focus_present_mask).astype(bool)
    w_qkv = np.asarray(w_qkv, dtype=np.float32)
    w_out = np.asarray(w_out, dtype=np.float32)

    U, S, NQ_U, NQ_M = plan(mask)
    cpb0 = NCORES // len(U) if U else 0
    use_ag = USE_AG and cpb0 >= 2
    nc = _get_nc(NQ_U, NQ_M, cpb=cpb0, use_ag=use_ag, legalize=LEGALIZE)

    # host-side weight prep (exact: 0.125 is a power of two)
    w_qkv_s = w_qkv.copy()
    w_qkv_s[:, :HID] *= np.float32(0.125)
    wq_np = _kchunked(_to_bf16(w_qkv_s))
    wo8_np = np.ascontiguousarray(_to_bf16(w_out).reshape(H, D, DIM))

    in_maps = []
    core_meta = []
    cpb = NCORES // len(U) if U else 0
    QBS = min(512, NQ_U) if NQ_U else 0
    NQB = NQ_U // QBS if NQ_U else 0
    if U:
        R8 = H * NQB
        eye8_np = np.zeros((R8, R8 * D), dtype=np.float32)
        for r in range(R8):
            eye8_np[r, r * D : (r + 1) * D] = 1.0
        eye8_np = eye8_np.astype(BF16)
    if S:
        m_rows_x = np.concatenate([x[b] for b in S], axis=0)  # (len(S)*N, DIM)
        wo4_np = _kchunked(_to_bf16(w_out))
        wv4T_np = _kchunked(_to_bf16(np.ascontiguousarray(w_qkv[:, 2 * HID :].T)))
    for c in range(NCORES):
        im = {"w_qkv": wq_np, "w_out8": wo8_np}
        meta = {}
        if U:
            b_u = U[c // cpb]
            qs = (c % cpb) * NQ_U
            if not use_ag:
                im["xT"] = _kchunked(_to_bf16(x[b_u].T))
            im["xqT"] = _kchunked(_to_bf16(x[b_u, qs : qs + NQ_U].T))
            im["eye8"] = eye8_np
            if mask[b_u]:
                # eye-encoded exp(bias): 0 off-diagonal, 1 on it — the exact
                # one-hot attention of a masked batch
                eb = np.zeros((H, NQ_U, N), dtype=np.float32)
                rows = np.arange(NQ_U)
                eb[:, rows, qs + rows] = 1.0
            else:
                # exp(sim + bias) = exp(sim) * exp(bias)
                eb = np.exp(pos_bias[:, qs : qs + NQ_U, :], dtype=np.float32)
            ebT = eb.transpose(0, 2, 1)  # (H, N, NQ_U)
            ebT = ebT.reshape(H, N, NQB, QBS).transpose(0, 2, 1, 3)
            im["ebT"] = np.ascontiguousarray(ebT.astype(BF16))
            meta["u"] = (b_u, qs)
        if S:
            ms = c * NQ_M
            im["xmT"] = _kchunked(_to_bf16(m_rows_x[ms : ms + NQ_M].T))
            im["w_out4"] = wo4_np
            im["wv4T"] = wv4T_np
            meta["m"] = ms
        in_maps.append(im)
        core_meta.append(meta)

    return nc, in_maps, core_meta, (U, S, NQ_U, NQ_M)


def gather(results, core_meta, plan_info):
    """Assemble per-core outputs into the full (B, N, DIM) array."""
    U, S, NQ_U, NQ_M = plan_info
    out = np.empty((B, N, DIM), dtype=np.float32)
    m_rows = np.empty((len(S) * N, DIM), dtype=np.float32) if S else None
    for c in range(NCORES):
        r = results[c]
        meta = core_meta[c]
        if "u" in meta:
            b_u, qs = meta["u"]
            out[b_u, qs : qs + NQ_U] = r["out_u"]
        if "m" in meta:
            ms = meta["m"]
            m_rows[ms : ms + NQ_M] = r["out_mT"].T
    for i, b in enumerate(S):
        out[b] = m_rows[i * N : (i + 1) * N]
    return out


def kernel(x, pos_bias, focus_present_mask, w_qkv, w_out, **run_kwargs):
    from concourse.bass_utils import run_bass_kernel_spmd

    nc, in_maps, core_meta, plan_info = prepare(
        x, pos_bias, focus_present_mask, w_qkv, w_out
    )
    res = run_bass_kernel_spmd(
        nc, in_maps, core_ids=list(range(NCORES)), **run_kwargs
    )
    out = gather(res.results, core_meta, plan_info)
    kernel.last_result = res
    return out


# revision 62
# speedup vs baseline: 1.0681x; 1.0681x over previous
"""Distributed Trainium2 Bass kernel for nn_Attention_15479062135535.

Reference computation (B=4, N=2048, DIM=512, H=8, D=64):
    qkv = x @ w_qkv; q,k,v = split(qkv)
    sim = (q * D**-.5) @ k^T + pos_bias
    masked batches (focus_present_mask): sim = -inf off-diagonal -> attn == eye
    out = softmax(sim) @ v @ w_out

Sharding (8 cores, no collectives):
  - Batches NOT fully masked ("attention batches", list U): each core owns one
    batch's contiguous block of NQ_U query rows x all 8 heads. K/V are
    recomputed per core from the full x[b] (cheap vs attention).
  - Fully masked batches (list S): attn == identity exactly, so
    out = (x @ w_v) @ w_out row-wise. Rows of all S batches are concatenated
    and split evenly across cores as a cheap passthrough stage.
  - |U| must divide 8; odd mask counts are rounded down (the leftover masked
    batch goes through the attention path with an eye-encoded bias, which is
    exact as well).

Numerics:
  - All matmuls bf16 with f32 PSUM accumulation.
  - softmax without max subtraction (|sim| <~ 8, exp is safe in f32).
  - exp(sim + bias) = exp(sim) * exp(bias): exp(pos_bias) is precomputed on
    host in f32 and passed as bf16; for masked rows it is the exact 0/1
    one-hot, so identity attention is exact.
  - softmax denominator: a ones-column appended to V makes the AV matmul
    accumulate sum(exp) in PSUM partition 64 for free.

The single SPMD program is specialized at build time on (NQ_U, NQ_M) only;
all per-core differences are in the input data.
"""

import sys

sys.path.insert(0, "/opt/trn_rl_repo")

import numpy as np
import ml_dtypes

B, N, DIM = 4, 2048, 512
H, D = 8, 64
HID = H * D
P = 128
NCORES = 8

BF16 = ml_dtypes.bfloat16

LEGALIZE = True  # sim_check sets False (CoreSim rejects the synthetic NoOps)
USE_AG = False  # AllGather K/V sharing: correctness+perf negative on this runtime

_NC_CACHE = {}


def _legalize_waits(nc, max_waits=1):
    """Split multi-wait sync_info into standalone NoOp waits.

    The walrus build in this container supports only one sync-wait command
    per instruction ("Too many sync wait commands" in setupSyncWait), while
    Tile embeds the full wait list in each instruction. Hoisting the extra
    waits onto engine-tagged NoOps immediately before the instruction is
    semantically identical (the engine stalls on each in turn).
    """
    import concourse.mybir as mybir
    import bass_rust

    ctr = 0
    for fn in nc.m.functions:
        for blk in fn.blocks:
            out = []
            changed = False
            for inst in blk.instructions:
                si = inst.sync_info
                if si is not None and si.on_wait and len(si.on_wait) > max_waits:
                    waits = list(si.on_wait)
                    for w in waits[:-max_waits]:
                        ctr += 1
                        nop = mybir.InstNoOp(name=f"waitnop-{ctr}")
                        nop.engine = inst.engine
                        nop.sync_info = bass_rust.SyncInfo(on_wait=[w], on_update=[])
                        out.append(nop)
                    inst.sync_info = bass_rust.SyncInfo(
                        on_wait=waits[-max_waits:], on_update=list(si.on_update)
                    )
                    changed = True
                out.append(inst)
            if changed:
                blk.instructions = out
    return ctr


def _build(NQ_U: int, NQ_M: int, cpb: int = 0, use_ag: bool = False):
    """Build the uniform SPMD Bass program for the given per-core row counts."""
    import concourse.bass as bass
    import concourse.mybir as mybir
    import concourse.tile as tile

    f32 = mybir.dt.float32
    bf16 = mybir.dt.bfloat16
    Exp = mybir.ActivationFunctionType.Exp

    nc = bass.Bass()

    w_qkv = nc.declare_dram_parameter("w_qkv", (P, 4, 3 * HID), bf16, isOutput=False)
    w_out8 = nc.declare_dram_parameter("w_out8", (H, D, DIM), bf16, isOutput=False)

    if NQ_U:
        QBS = min(512, NQ_U)  # q-block size (free dim of sim matmuls)
        NQB = NQ_U // QBS
        KCP = 4  # key-chunks per psum tile / exp batch
        use_ag = use_ag and cpb >= 2
        if not use_ag:
            xT = nc.declare_dram_parameter("xT", (P, 4, N), bf16, isOutput=False)
        xqT = nc.declare_dram_parameter("xqT", (P, 4, NQ_U), bf16, isOutput=False)
        # ebT = exp(pos_bias)^T slabs: exp(sim+bias) = exp(sim)*exp(bias)
        ebT = nc.declare_dram_parameter(
            "ebT", (H, NQB, N, QBS), bf16, isOutput=False
        )
        # eye8[r, r*64:(r+1)*64] == 1: lhsT selector that broadcasts row r of
        # an (R, n) rhs onto 64 output partitions
        R8 = H * NQB
        eye8 = nc.declare_dram_parameter("eye8", (R8, R8 * D), bf16, isOutput=False)
        out_u = nc.declare_dram_parameter("out_u", (NQ_U, DIM), f32, isOutput=True)
    if NQ_M:
        MBS = 256  # masked-stage column chunk (NQ_M is a multiple of 256)
        xmT = nc.declare_dram_parameter("xmT", (P, 4, NQ_M), bf16, isOutput=False)
        w_out4 = nc.declare_dram_parameter("w_out4", (P, 4, DIM), bf16, isOutput=False)
        wv4T = nc.declare_dram_parameter("wv4T", (P, 4, DIM), bf16, isOutput=False)
        out_mT = nc.declare_dram_parameter("out_mT", (DIM, NQ_M), f32, isOutput=True)

    with tile.TileContext(nc) as tc:
        with (
            tc.tile_pool(name="const", bufs=1) as const,
            tc.tile_pool(name="expbp", bufs=2) as expbp,
            tc.tile_pool(name="attnp", bufs=4) as attnp,
            tc.tile_pool(name="outp", bufs=3) as outp,
            tc.tile_pool(name="small", bufs=3) as small,
            tc.tile_pool(name="ps_p", bufs=3, space="PSUM") as ps_big,
            tc.tile_pool(name="ps_o", bufs=2, space="PSUM") as ps_o,
        ):
            ps_sim = ps_big  # shared pool: all tiles are one PSUM slot size
            # ---- weights: column-chunked DMAs so consumers start early ----
            wq_sb = const.tile([P, 4, 3 * HID], bf16, name="wq_sb")

            def load_wq(c0, c1):
                nc.sync.dma_start(
                    wq_sb[:, :, c0 * 128 : c1 * 128], w_qkv[:, :, c0 * 128 : c1 * 128]
                )

            if NQ_U:
                # ---- stage A: activations ----
                xqT_sb = const.tile([P, 4, NQ_U], bf16, name="xqT_sb")
                nc.sync.dma_start(xqT_sb[:], xqT[:])
                if use_ag:
                    load_wq(4, 12)  # k+v columns (KV shards need them first)
                    load_wq(0, 4)  # q columns
                else:
                    load_wq(0, 4)  # q columns
                    load_wq(4, 8)  # k columns
                    xT_sb = const.tile([P, 4, N], bf16, name="xT_sb")
                    for n in range(4):
                        nc.sync.dma_start(
                            xT_sb[:, :, n * 512 : (n + 1) * 512],
                            xT[:, :, n * 512 : (n + 1) * 512],
                        )
                    load_wq(8, 12)  # v columns
                eye8_sb = const.tile([R8, R8 * D], bf16, name="eye8_sb")
                nc.sync.dma_start(eye8_sb[:], eye8[:])
            else:
                load_wq(0, 12)

            wo8_sb = []
            for h in range(H):
                t = const.tile([D, DIM], bf16, name=f"wo8_{h}")
                nc.sync.dma_start(t[:], w_out8[h])
                wo8_sb.append(t)

            if NQ_M:
                # ---- passthrough: out_m = x_m @ (w_v @ w_out). W2 = w_v@w_out
                # is computed once on device; out_m groups are emitted as
                # fillers inside stage C so their matmuls use PE idle time ----
                xmT_sb = const.tile([P, 4, NQ_M], bf16, name="xmT_sb")
                nc.sync.dma_start(xmT_sb[:], xmT[:])
                wo4_sb = const.tile([P, 4, DIM], bf16, name="wo4_sb")
                nc.sync.dma_start(wo4_sb[:], w_out4[:])
                wv4T_sb = const.tile([P, 4, DIM], bf16, name="wv4T_sb")
                nc.sync.dma_start(wv4T_sb[:], wv4T[:])
                w2_sb = const.tile([P, 4, DIM], bf16, name="w2_sb")
                MCHUNKS = []
                o = 0
                while o < NQ_M:
                    w = min(512, NQ_M - o)
                    MCHUNKS.append((o, w))
                    o += w
                NMC = len(MCHUNKS)

                def emit_w2(dc):
                    # W2[dc-chunk] = w_v @ w_out (contraction over hid):
                    # lhsT = w_v^T chunk (hid, d), rhs = w_out (hid, out)
                    ps = ps_big.tile([P, 512], f32, tag="ps")
                    for kc in range(4):
                        nc.tensor.matmul(
                            ps[:],
                            wv4T_sb[:, kc, dc * 128 : (dc + 1) * 128],
                            wo4_sb[:, kc, :],
                            start=(kc == 0),
                            stop=(kc == 3),
                        )
                    nc.vector.tensor_copy(w2_sb[:, dc, :], ps[:])

                def emit_masked_out(mo, n):
                    o, w = MCHUNKS[n]
                    nsl = slice(o, o + w)
                    ps = ps_big.tile([P, 512], f32, tag="ps")
                    for kc in range(4):
                        nc.tensor.matmul(
                            ps[:, :w],
                            w2_sb[:, kc, mo * 128 : (mo + 1) * 128],
                            xmT_sb[:, kc, nsl],
                            start=(kc == 0),
                            stop=(kc == 3),
                        )
                    ot = outp.tile([P, 512], f32, tag="otm")
                    nc.vector.tensor_copy(ot[:, :w], ps[:, :w])
                    nc.sync.dma_start(out_mT[mo * 128 : (mo + 1) * 128, nsl], ot[:, :w])

            if NQ_U:
                # ---- stage B: q^T, k^T (d on partitions, head pairs stacked),
                # v (+ones col); emitted partly as fillers inside stage C ----
                qT2 = [const.tile([P, NQ_U], bf16, name=f"qT2_{hp}") for hp in range(4)]
                kT2 = [const.tile([P, N], bf16, name=f"kT2_{hp}") for hp in range(4)]
                v_sb = const.tile([P, 16, H, D + 1], bf16, name="v_sb")
                if not use_ag:
                    nc.vector.memset(v_sb[:, :, :, D], 1.0)

                def emit_Q(mq):
                    for n in range(NQ_U // QBS):
                        ps = ps_big.tile([P, 512], f32, tag="ps")
                        for kc in range(4):
                            nc.tensor.matmul(
                                ps[:, :QBS],
                                wq_sb[:, kc, mq * 128 : (mq + 1) * 128],
                                xqT_sb[:, kc, n * QBS : (n + 1) * QBS],
                                start=(kc == 0),
                                stop=(kc == 3),
                            )
                        nc.scalar.copy(qT2[mq][:, n * QBS : (n + 1) * QBS], ps[:, :QBS])

                def emit_K(mk, n):
                    ps = ps_big.tile([P, 512], f32, tag="ps")
                    for kc in range(4):
                        nc.tensor.matmul(
                            ps[:],
                            wq_sb[:, kc, HID + mk * 128 : HID + (mk + 1) * 128],
                            xT_sb[:, kc, n * 512 : (n + 1) * 512],
                            start=(kc == 0),
                            stop=(kc == 3),
                        )
                    if n % 2 == 0:
                        nc.scalar.copy(kT2[mk][:, n * 512 : (n + 1) * 512], ps[:])
                    else:
                        nc.vector.tensor_copy(kT2[mk][:, n * 512 : (n + 1) * 512], ps[:])

                def emit_V(t):
                    ps = ps_big.tile([P, 512], f32, tag="ps")
                    for kc in range(4):
                        nc.tensor.matmul(
                            ps[:],
                            xT_sb[:, kc, t * 128 : (t + 1) * 128],
                            wq_sb[:, kc, 2 * HID : 3 * HID],
                            start=(kc == 0),
                            stop=(kc == 3),
                        )
                    nc.vector.tensor_copy(
                        v_sb[:, t, :, 0:D],
                        ps[:].rearrange("p (h d) -> p h d", h=H),
                    )

                if use_ag:
                    # ---- KV shards + AllGather: each core computes K/V for
                    # its own x slice (== its q rows) and the cpb cores of a
                    # batch group exchange shards via AllGather ----
                    NKV = N // cpb  # kv columns produced per core
                    TKV = NKV // 128
                    VW = H * (D + 1)
                    KSZ = 4 * P * NKV
                    VSZ = TKV * P * VW
                    with tc.tile_pool(name="dramp", bufs=1, space="DRAM") as dramp:
                        ag_in = dramp.tile([KSZ + VSZ], bf16, name="ag_in")
                        ag_out = dramp.tile([cpb * (KSZ + VSZ)], bf16, name="ag_out")
                        kstage = const.tile([P, 4, NKV], bf16, name="kstage")
                        vstage = const.tile([P, TKV, VW], bf16, name="vstage")
                        nc.vector.memset(
                            vstage[:].rearrange("p t (h e) -> p t h e", h=H)[
                                :, :, :, D
                            ],
                            1.0,
                        )
                        for hp in range(4):
                            for n0 in range(0, NKV, 512):
                                nw = min(512, NKV - n0)
                                ps = ps_big.tile([P, 512], f32, tag="ps")
                                for kc in range(4):
                                    nc.tensor.matmul(
                                        ps[:, :nw],
                                        wq_sb[:, kc, HID + hp * 128 : HID + (hp + 1) * 128],
                                        xqT_sb[:, kc, n0 : n0 + nw],
                                        start=(kc == 0),
                                        stop=(kc == 3),
                                    )
                                nc.scalar.copy(
                                    kstage[:, hp, n0 : n0 + nw], ps[:, :nw]
                                )
                        for t in range(TKV):
                            ps = ps_big.tile([P, 512], f32, tag="ps")
                            for kc in range(4):
                                nc.tensor.matmul(
                                    ps[:],
                                    xqT_sb[:, kc, t * 128 : (t + 1) * 128],
                                    wq_sb[:, kc, 2 * HID : 3 * HID],
                                    start=(kc == 0),
                                    stop=(kc == 3),
                                )
                            nc.vector.tensor_copy(
                                vstage[:, t].rearrange("p (h e) -> p h e", h=H)[
                                    :, :, 0:D
                                ],
                                ps[:].rearrange("p (h d) -> p h d", h=H),
                            )
                        k_dst = ag_in[0:KSZ].rearrange(
                            "(hp r) -> hp r", hp=4
                        ).rearrange("hp (p q) -> p hp q", p=P)
                        assert tuple(k_dst.shape) == (P, 4, NKV), k_dst.shape
                        nc.sync.dma_start(k_dst, kstage[:])
                        v_dst = ag_in[KSZ:].rearrange(
                            "(t r) -> t r", t=TKV
                        ).rearrange("t (p f) -> p t f", p=P)
                        assert tuple(v_dst.shape) == (P, TKV, VW), v_dst.shape
                        nc.sync.dma_start(v_dst, vstage[:])
                        groups = [
                            [g * cpb + i for i in range(cpb)]
                            for g in range(NCORES // cpb)
                        ]
                        nc.gpsimd.collective_compute(
                            "AllGather",
                            mybir.AluOpType.bypass,
                            replica_groups=groups,
                            ins=[ag_in.opt()],
                            outs=[ag_out.opt()],
                        )
                        agv = ag_out[:].rearrange("(c x) -> c x", c=cpb)
                        for hp in range(4):
                            src_k = agv[
                                :, hp * P * NKV : (hp + 1) * P * NKV
                            ].rearrange("c (p q) -> c p q", p=P)
                            assert tuple(src_k.shape) == (cpb, P, NKV), src_k.shape
                            k_gd = kT2[hp][:].rearrange("p (c q) -> c p q", q=NKV)
                            assert tuple(k_gd.shape) == (cpb, P, NKV), k_gd.shape
                            nc.sync.dma_start(k_gd, src_k)
                        v_flat = v_sb[:].rearrange("p t h e -> p t (h e)")
                        for c in range(cpb):
                            off = c * (KSZ + VSZ) + KSZ
                            v_src = ag_out[off : off + VSZ].rearrange(
                                "(t r) -> t r", t=TKV
                            ).rearrange("t (p f) -> p t f", p=P)
                            assert tuple(v_src.shape) == (P, TKV, VW), v_src.shape
                            nc.sync.dma_start(
                                v_flat[:, c * TKV : (c + 1) * TKV, :], v_src
                            )
                    emit_Q(0)
                else:
                    emit_Q(0)
                    for n in range(4):
                        emit_K(0, n)
                    for t in range(16):
                        emit_V(t)

                # fillers: (deadline_head, fn, args) — kq fillers must be
                # emitted before the head that consumes them; masked fillers
                # have no deadline
                fillers = []
                for hp in range(1, 4):
                    fillers.append((2 * hp, emit_Q, (hp,)))
                    if not use_ag:
                        for n in range(4):
                            fillers.append((2 * hp, emit_K, (hp, n)))
                if NQ_M:
                    for mo in range(4):
                        fillers.append((99, emit_w2, (mo,)))
                    for n in range(NMC):
                        for mo in range(4):
                            fillers.append((99, emit_masked_out, (mo, n)))
                fillers.reverse()  # pop from the end

                # ---- stage C: attention per (head, q-block) ----
                # simT psum = q@k^T; exp on the scalar engine straight out of
                # PSUM; attn = exp(sim)*exp(bias) on the vector engine; AV
                # matmul with a ones-column in V accumulates the softmax
                # denominator for free.
                aoU = [const.tile([D, NQ_U], bf16, name=f"aoU{h}") for h in range(H)]
                aoT = [const.tile([D, NQ_U], bf16, name=f"aoT{h}") for h in range(H)]
                den8 = const.tile([H * NQB, QBS], f32, name="den8")
                NKP = 16 // KCP  # psum tiles per (h, qb)
                for h in range(H):
                    hp, hl = h // 2, (h % 2) * D
                    while fillers and fillers[-1][0] <= h + 2:
                        _, f, a = fillers.pop()
                        f(*a)
                    for qb in range(NQB):
                        qsl = slice(qb * QBS, (qb + 1) * QBS)
                        slab = expbp.tile([P, 16, QBS], bf16, tag="slab")
                        nc.sync.dma_start(
                            slab[:],
                            ebT[h, qb].rearrange("(kc p) q -> p kc q", p=P),
                        )
                        ps_o_t = ps_o.tile([D + 1, QBS], f32, tag="ps_o")
                        for kp in range(NKP):
                            ps_t = ps_sim.tile([P, KCP, QBS], f32, tag="ps")
                            for i in range(KCP):
                                kc = kp * KCP + i
                                nc.tensor.matmul(
                                    ps_t[:, i, :],
                                    kT2[hp][hl : hl + D, kc * 128 : (kc + 1) * 128],
                                    qT2[hp][hl : hl + D, qsl],
                                    start=True,
                                    stop=True,
                                )
                            at = attnp.tile([P, KCP, QBS], bf16, tag="at")
                            nc.scalar.activation(at[:], ps_t[:], Exp)
                            atm = attnp.tile([P, KCP, QBS], bf16, tag="atm")
                            nc.vector.tensor_mul(
                                atm[:], at[:], slab[:, kp * KCP : (kp + 1) * KCP, :]
                            )
                            for i in range(KCP):
                                kc = kp * KCP + i
                                nc.tensor.matmul(
                                    ps_o_t[:],
                                    v_sb[:, kc, h, :],
                                    atm[:, i, :],
                                    start=(kc == 0),
                                    stop=(kc == 15),
                                )
                            if kp % 2 == 0 and len(fillers) > 8:
                                _, f, a = fillers.pop()
                                f(*a)
                        nc.scalar.copy(aoU[h][:, qsl], ps_o_t[0:D, :])
                        r = h * NQB + qb
                        d1 = small.tile([1, QBS], f32, tag="d1")
                        nc.vector.tensor_copy(d1[:], ps_o_t[D : D + 1, :])
                        nc.gpsimd.dma_start(den8[r : r + 1, :], d1[:])

                for _ in range(min(5, len(fillers))):
                    _, f, a = fillers.pop()
                    f(*a)

                # ---- batched softmax normalization ----
                recip8 = small.tile([H * NQB, QBS], f32, tag="recip8")
                nc.vector.reciprocal(recip8[:], den8[:])
                recip8_bf = small.tile([H * NQB, QBS], bf16, tag="recip8b")
                nc.vector.tensor_copy(recip8_bf[:], recip8[:])
                for h in range(H):
                    for qb in range(NQB):
                        qsl = slice(qb * QBS, (qb + 1) * QBS)
                        r = h * NQB + qb
                        ps_bt = ps_o.tile([D + 1, QBS], f32, tag="ps_o")
                        nc.tensor.matmul(
                            ps_bt[0:D, :],
                            eye8_sb[:, r * D : (r + 1) * D],
                            recip8_bf[:],
                            start=True,
                            stop=True,
                        )
                        rb = small.tile([D, QBS], f32, tag="rb")
                        nc.scalar.copy(rb[:], ps_bt[0:D, :])
                        nc.vector.tensor_mul(aoT[h][:, qsl], aoU[h][:, qsl], rb[:])
                        if fillers:
                            _, f, a = fillers.pop()
                            f(*a)

                while fillers:
                    _, f, a = fillers.pop()
                    f(*a)

                # ---- stage D: out projection ----
                for t in range(NQ_U // 128):
                    ps = ps_big.tile([P, 512], f32, tag="ps")
                    for h in range(H):
                        nc.tensor.matmul(
                            ps[:],
                            aoT[h][:, t * 128 : (t + 1) * 128],
                            wo8_sb[h][:],
                            start=(h == 0),
                            stop=(h == 7),
                        )
                    ot = outp.tile([P, 512], f32)
                    nc.vector.tensor_copy(ot[:], ps[:])
                    nc.sync.dma_start(out_u[t * 128 : (t + 1) * 128, :], ot[:])

            if NQ_M and not NQ_U:
                for mo in range(4):
                    emit_w2(mo)
                for n in range(NMC):
                    for mo in range(4):
                        emit_masked_out(mo, n)

    return nc


def _get_nc(NQ_U: int, NQ_M: int, cpb: int = 0, use_ag: bool = False,
            legalize: bool = True):
    key = (NQ_U, NQ_M, cpb, use_ag, legalize)
    if key not in _NC_CACHE:
        nc = _build(NQ_U, NQ_M, cpb=cpb, use_ag=use_ag)
        if legalize:
            _legalize_waits(nc)
        _NC_CACHE[key] = nc
    return _NC_CACHE[key]


def _to_bf16(a):
    return np.ascontiguousarray(np.asarray(a, dtype=np.float32)).astype(BF16)


def _kchunked(a2d):
    """(512, n) -> (128, 4, n): contraction dim split into 4 partition chunks."""
    k, n = a2d.shape
    assert k == 4 * P
    return np.ascontiguousarray(a2d.reshape(4, P, n).transpose(1, 0, 2))


def plan(mask):
    """Return (U, S, NQ_U, NQ_M, assignments) for the given bool mask."""
    mask = np.asarray(mask).astype(bool)
    masked = [b for b in range(B) if mask[b]]
    unmasked = [b for b in range(B) if not mask[b]]
    # |U| must divide 8 and be in {1,2,4}: move masked batches back into the
    # attention path (exact via eye-encoded bias) until it does.
    S = list(masked)
    U = list(unmasked)
    while len(U) not in (0, 1, 2, 4):
        U.append(S.pop())
    if len(U) == 0 and len(S) == 0:  # unreachable, B=4
        raise AssertionError
    NQ_U = (N * len(U)) // NCORES if U else 0
    NQ_M = (N * len(S)) // NCORES
    return U, S, NQ_U, NQ_M


def prepare(x, pos_bias, focus_present_mask, w_qkv, w_out):
    """Build (nc, in_maps, core_meta, plan_info) for the given inputs."""
    x = np.asarray(x, dtype=np.float32)
    pos_bias = np.asarray(pos_bias, dtype=np.float32)
    mask = np.asarray(focus_present_mask).astype(bool)
    w_qkv = np.asarray(w_qkv, dtype=np.float32)
    w_out = np.asarray(w_out, dtype=np.float32)

    U, S, NQ_U, NQ_M = plan(mask)
    cpb0 = NCORES // len(U) if U else 0
    use_ag = USE_AG and cpb0 >= 2
    nc = _get_nc(NQ_U, NQ_M, cpb=cpb0, use_ag=use_ag, legalize=LEGALIZE)

    # host-side weight prep (exact: 0.125 is a power of two)
    w_qkv_s = w_qkv.copy()
    w_qkv_s[:, :HID] *= np.float32(0.125)
    wq_np = _kchunked(_to_bf16(w_qkv_s))
    wo8_np = np.ascontiguousarray(_to_bf16(w_out).reshape(H, D, DIM))

    in_maps = []
    core_meta = []
    cpb = NCORES // len(U) if U else 0
    QBS = min(512, NQ_U) if NQ_U else 0
    NQB = NQ_U // QBS if NQ_U else 0
    if U:
        R8 = H * NQB
        eye8_np = np.zeros((R8, R8 * D), dtype=np.float32)
        for r in range(R8):
            eye8_np[r, r * D : (r + 1) * D] = 1.0
        eye8_np = eye8_np.astype(BF16)
    if S:
        m_rows_x = np.concatenate([x[b] for b in S], axis=0)  # (len(S)*N, DIM)
        wo4_np = _kchunked(_to_bf16(w_out))
        wv4T_np = _kchunked(_to_bf16(np.ascontiguousarray(w_qkv[:, 2 * HID :].T)))
    for c in range(NCORES):
        im = {"w_qkv": wq_np, "w_out8": wo8_np}
        meta = {}
        if U:
            b_u = U[c // cpb]
            qs = (c % cpb) * NQ_U
            if not use_ag:
                im["xT"] = _kchunked(_to_bf16(x[b_u].T))
            im["xqT"] = _kchunked(_to_bf16(x[b_u, qs : qs + NQ_U].T))
            im["eye8"] = eye8_np
            if mask[b_u]:
                # eye-encoded exp(bias): 0 off-diagonal, 1 on it — the exact
                # one-hot attention of a masked batch
                eb = np.zeros((H, NQ_U, N), dtype=np.float32)
                rows = np.arange(NQ_U)
                eb[:, rows, qs + rows] = 1.0
            else:
                # exp(sim + bias) = exp(sim) * exp(bias)
                eb = np.exp(pos_bias[:, qs : qs + NQ_U, :], dtype=np.float32)
            ebT = eb.transpose(0, 2, 1)  # (H, N, NQ_U)
            ebT = ebT.reshape(H, N, NQB, QBS).transpose(0, 2, 1, 3)
            im["ebT"] = np.ascontiguousarray(ebT.astype(BF16))
            meta["u"] = (b_u, qs)
        if S:
            ms = c * NQ_M
            im["xmT"] = _kchunked(_to_bf16(m_rows_x[ms : ms + NQ_M].T))
            im["w_out4"] = wo4_np
            im["wv4T"] = wv4T_np
            meta["m"] = ms
        in_maps.append(im)
        core_meta.append(meta)

    return nc, in_maps, core_meta, (U, S, NQ_U, NQ_M)


def gather(results, core_meta, plan_info):
    """Assemble per-core outputs into the full (B, N, DIM) array."""
    U, S, NQ_U, NQ_M = plan_info
    out = np.empty((B, N, DIM), dtype=np.float32)
    m_rows = np.empty((len(S) * N, DIM), dtype=np.float32) if S else None
    for c in range(NCORES):
        r = results[c]
        meta = core_meta[c]
        if "u" in meta:
            b_u, qs = meta["u"]
            out[b_u, qs : qs + NQ_U] = r["out_u"]
        if "m" in meta:
            ms = meta["m"]
            m_rows[ms : ms + NQ_M] = r["out_mT"].T
    for i, b in enumerate(S):
        out[b] = m_rows[i * N : (i + 1) * N]
    return out


def kernel(x, pos_bias, focus_present_mask, w_qkv, w_out, **run_kwargs):
    from concourse.bass_utils import run_bass_kernel_spmd

    nc, in_maps, core_meta, plan_info = prepare(
        x, pos_bias, focus_present_mask, w_qkv, w_out
    )
    res = run_bass_kernel_spmd(
        nc, in_maps, core_ids=list(range(NCORES)), **run_kwargs
    )
    out = gather(res.results, core_meta, plan_info)
    kernel.last_result = res
    return out
